# revision 39
# baseline (speedup 1.0000x reference)
"""ConvLSTM Trainium2 kernel (8 NeuronCores, SPMD).

Reference computation:
  phase 1: y = conv3x3(x, Wx) + bx  over all T*N frames,
           BatchNorm(train, biased var) over (T*N, H, W) per channel,
           y = y_hat * gamma + beta
  phase 2: per t: tmp = conv3x3(h, Wh) + y[t]; i,f,o = sigmoid, g = tanh
           c = f*c + i*g ; h = o*tanh(c)
  output hs: [T, N, 64, H, W]

Key algebra used here:
  - the conv bias bx cancels exactly inside BatchNorm (mean removes it),
    so bx is never used.
  - BN is folded to y_norm = y0*scale_c + shift_c with
    scale = gamma*rsqrt(var+eps), shift = beta - mean*scale, where y0 is
    the raw (bias-free) conv output.  scale is applied by an extra
    "diag(scale)" matmul accumulated into the same PSUM group as the h2h
    conv; shift is folded into the gate activation's per-partition bias.

Distribution (default, exchange=True): core c handles batch n = c//2 and
H-half s = c%2 (top/bottom 32 rows).  Phase 1 needs no communication (x
arrives host-pre-padded with halo rows); the recurrence swaps one halo
row of h per step with the pair partner via a 2-rank AllGather, masked
per-core so global edges stay zero.  BN statistics are summed across all
8 cores with one tiny AllReduce.  exchange=False falls back to pairs
duplicating the full-H recurrence (no per-step collectives).

Conv-as-matmul: 3x3 taps become 6 matmuls per 128-out-channel group per
512-pixel chunk: 3 "pair" matmuls (kh=0 and kh=2 packed along K=128 via a
row-shifted copy of the input living in SBUF partitions 64..127) plus 3
"single" matmuls (kh=1, K=64).
"""

import os
import numpy as np
import ml_dtypes

import concourse.bass as bass
import concourse.bacc as bacc
import concourse.mybir as mybir
from concourse.tile import TileContext
from concourse.bass_utils import run_bass_kernel_spmd

BF16 = mybir.dt.bfloat16
F32 = mybir.dt.float32
AF = mybir.ActivationFunctionType
OP = mybir.AluOpType

BN_EPS = 1e-5
CIN = 64   # conv input channels (also hidden size)
NG = 2     # output-channel groups of 128 (4*64 = 256 = 2*128)


def _slot(conv, g, kind, kw):
    """Weight slot index in the packed [128, 24, 128] lhsT tensor."""
    return conv * 12 + g * 6 + (kw if kind == "pair" else 3 + kw)


def build(T=16, HL=64, W=64, n_cores=8, exchange=False, pool_bufs=2, repeat=1,
          skip_cc=False, ph1=True, ph2=True):
    # skip_cc=True omits the per-step halo AllGather (WRONG results;
    # timing-only, to isolate collective cost).
    # HL = rows of H owned by this core.  exchange=False: every core holds
    # the full H (pairs duplicate phase-2 work).  exchange=True: pairs
    # split H in half and swap one halo row per step via a 2-rank
    # AllGather.
    RP, WP = HL + 2, W + 2         # local padded rows / cols
    L = RP * WP                    # flat padded length
    CHR = 8                        # rows per chunk
    assert HL % CHR == 0
    NCH = HL // CHR                # chunks per local frame
    CPX = CHR * W                  # pixels per chunk (<= 512)
    assert CPX <= 512
    PX = HL * W
    NCOL = T * NCH                 # stat columns per group

    nc = bacc.Bacc(num_devices=n_cores)

    x_ext = nc.declare_dram_parameter("x", [T, CIN, RP, WP], BF16, isOutput=False)
    w_ext = nc.declare_dram_parameter("w", [128, 24, 128], BF16, isOutput=False)
    gb_ext = nc.declare_dram_parameter("gb", [128, 4], F32, isOutput=False)
    id_ext = nc.declare_dram_parameter("ident", [128, 128], BF16, isOutput=False)
    hm_ext = nc.declare_dram_parameter("hm", [128, 2], F32, isOutput=False)
    out_ext = nc.declare_dram_parameter("out", [T, CIN, PX], BF16, isOutput=True)

    y0_dram = nc.dram_tensor("y0_buf", [T, 128, NG, PX], BF16)
    cc_in = nc.dram_tensor("cc_in", [128, 4], F32)
    cc_out = nc.dram_tensor("cc_out", [128, 4], F32, addr_space="Shared")
    if exchange:
        # double-buffered halo bounce buffers (one pair per step parity)
        cch_in = [nc.dram_tensor(f"cch_in{i}", [64, 2 * W], BF16) for i in range(2)]
        cch_out = [nc.dram_tensor(f"cch_out{i}", [128, 2 * W], BF16) for i in range(2)]
        pair_groups = [[2 * i, 2 * i + 1] for i in range(n_cores // 2)]

    def conv_mms(ps, pad_tile, conv, g, r0, nr, first_start):
        """6 matmuls accumulating conv tap contributions for rows r0..r0+nr."""
        v = pad_tile[:].rearrange("p (r c) -> p r c", c=WP)
        taps = [("pair", 0), ("pair", 1), ("pair", 2),
                ("single", 0), ("single", 1), ("single", 2)]
        for idx, (kind, kw) in enumerate(taps):
            s = _slot(conv, g, kind, kw)
            if kind == "pair":
                lhsT = w_sb[:, s, :]
                rhs = v[:, r0:r0 + nr, kw:kw + W]
            else:
                lhsT = w_sb[0:64, s, :]
                rhs = v[0:64, r0 + 1:r0 + 1 + nr, kw:kw + W]
            nc.tensor.matmul(
                ps, lhsT, rhs,
                start=(first_start and idx == 0),
                stop=(idx == len(taps) - 1),
            )

    with TileContext(nc) as tc:
        with (
            tc.tile_pool(name="const", bufs=1) as constp,
            tc.tile_pool(name="state", bufs=1) as statep,
            tc.tile_pool(name="io", bufs=pool_bufs) as iop,
            tc.tile_pool(name="cc", bufs=pool_bufs) as ccp,
            tc.tile_pool(name="work", bufs=pool_bufs) as workp,
            tc.tile_pool(name="psum", bufs=8, space="PSUM") as psump,
        ):
            # ---- constants ----
            w_sb = constp.tile([128, 24, 128], BF16, tag="w_sb", name="w_sb")
            ident_sb = constp.tile([128, 128], BF16, tag="ident_sb", name="ident_sb")
            gb_sb = constp.tile([128, 4], F32, tag="gb_sb", name="gb_sb")
            hm_sb = constp.tile([128, 2], F32, tag="hm_sb", name="hm_sb")
            nc.sync.dma_start(out=w_sb[:], in_=w_ext[:])
            nc.sync.dma_start(out=ident_sb[:], in_=id_ext[:])
            nc.sync.dma_start(out=gb_sb[:], in_=gb_ext[:])
            nc.sync.dma_start(out=hm_sb[:], in_=hm_ext[:])

            # ---- persistent state ----
            x_pads = [statep.tile([128, L], BF16, tag=f"x_pad{i}", name=f"x_pad{i}") for i in range(2)]
            h_pads = [statep.tile([128, L], BF16, tag=f"h_pad{i}", name=f"h_pad{i}") for i in range(2)]
            c_sb = statep.tile([64, PX], F32, tag="c_sb", name="c_sb")
            sum_cols = statep.tile([128, NG, NCOL], F32, tag="sum_cols", name="sum_cols")
            sq_cols = statep.tile([128, NG, NCOL], F32, tag="sq_cols", name="sq_cols")
            s1 = statep.tile([128, NG], F32, tag="s1", name="s1")
            s2 = statep.tile([128, NG], F32, tag="s2", name="s2")
            mean = statep.tile([128, NG], F32, tag="mean", name="mean")
            ex2 = statep.tile([128, NG], F32, tag="ex2", name="ex2")
            m2 = statep.tile([128, NG], F32, tag="m2", name="m2")
            var = statep.tile([128, NG], F32, tag="var", name="var")
            std = statep.tile([128, NG], F32, tag="std", name="std")
            rstd = statep.tile([128, NG], F32, tag="rstd", name="rstd")
            scalef = statep.tile([128, NG], F32, tag="scalef", name="scalef")
            msc = statep.tile([128, NG], F32, tag="msc", name="msc")
            shift = statep.tile([128, NG], F32, tag="shift", name="shift")
            red = statep.tile([128, 4], F32, tag="red", name="red")
            diag_sb = statep.tile([128, NG, 128], BF16, tag="diag_sb", name="diag_sb")

            # repeat>1 re-runs the whole computation back-to-back in one
            # launch -- measurement-only (exec time = slope vs repeat);
            # outputs are still those of the last rep's recurrence.
            def _body():
                # Zero h_pads on DVE (not gpsimd.memset): the first matmuls
                # reading h_pad already wait on DVE, so this adds no extra
                # sync-wait (MMs allow at most 3).
                zero_l = nc.const_aps.tensor(0.0, (128, L), F32)
                for tile_ in h_pads:
                    nc.vector.tensor_copy(out=tile_[:], in_=zero_l)

                # =================== phase 1: x2h conv + stats ===================
                for t in range(T if ph1 else 0):
                    xp = x_pads[t % 2]
                    xv = xp[:].rearrange("p (r c) -> p r c", c=WP)
                    # x arrives pre-padded [CIN, RP, WP]; base copy fills the
                    # whole tile (borders included), so no on-device memset.
                    nc.sync.dma_start(out=xv[0:64, :, :], in_=x_ext[t])
                    # row-shift(+2) copy: p64 row r = padded row r+2
                    nc.sync.dma_start(out=xv[64:128, 0:HL, :], in_=x_ext[t, :, 2:HL + 2, :])

                    y0s = iop.tile([128, NG, PX], BF16, tag="y0t", name="y0t")
                    for g in range(NG):
                        for j in range(NCH):
                            ps = psump.tile([128, CPX], F32, tag="ps", name="ps")
                            conv_mms(ps[:], xp, 0, g, j * CHR, CHR, first_start=True)
                            col = t * NCH + j
                            # psum -> bf16 y0 slice, plus channel sum (accum_out)
                            nc.vector.tensor_scalar(
                                out=y0s[:, g, j * CPX:(j + 1) * CPX],
                                in0=ps[:], scalar1=1.0, scalar2=0.0,
                                op0=OP.mult, op1=OP.add,
                                accum_out=sum_cols[:, g, col:col + 1],
                            )
                            # channel sum of squares
                            scr = workp.tile([128, CPX], BF16, tag="sqscr", name="sqscr")
                            nc.scalar.activation(
                                out=scr[:], in_=ps[:], func=AF.Square,
                                accum_out=sq_cols[:, g, col:col + 1],
                            )
                    nc.sync.dma_start(out=y0_dram[t], in_=y0s[:])

                # =================== BN stats -> scale/shift ===================
                if not ph1:
                    # timing-only stub (ph2-only build): unit scale, zero shift
                    nc.vector.tensor_scalar_mul(out=shift[:], in0=gb_sb[:, 2:4],
                                                scalar1=0.0)
                    for g in range(NG):
                        nc.vector.tensor_copy(out=diag_sb[:, g, :], in_=ident_sb[:])
                else:
                    nc.vector.tensor_reduce(out=s1[:], in_=sum_cols[:],
                                            axis=mybir.AxisListType.X, op=OP.add)
                    nc.vector.tensor_reduce(out=s2[:], in_=sq_cols[:],
                                            axis=mybir.AxisListType.X, op=OP.add)
                    nc.sync.dma_start(out=cc_in[:, 0:2], in_=s1[:])
                    nc.sync.dma_start(out=cc_in[:, 2:4], in_=s2[:])
                    nc.gpsimd.collective_compute(
                        "AllReduce", OP.add,
                        replica_groups=[list(range(n_cores))],
                        ins=[cc_in[:]], outs=[cc_out[:]],
                    )
                    nc.sync.dma_start(out=red[:], in_=cc_out[:])
                    inv = 1.0 / float(n_cores * T * HL * W)
                    nc.vector.tensor_scalar_mul(out=mean[:], in0=red[:, 0:2], scalar1=inv)
                    nc.vector.tensor_scalar_mul(out=ex2[:], in0=red[:, 2:4], scalar1=inv)
                    nc.vector.tensor_tensor(out=m2[:], in0=mean[:], in1=mean[:], op=OP.mult)
                    # var+eps = (ex2 + eps) - mean^2, fused in one op
                    nc.vector.scalar_tensor_tensor(out=var[:], in0=ex2[:], scalar=BN_EPS,
                                                   in1=m2[:], op0=OP.add, op1=OP.subtract)
                    nc.scalar.activation(out=std[:], in_=var[:], func=AF.Sqrt)
                    # ~18-bit 1/x, ~5x faster than InstReciprocal; var+eps is
                    # O(1) here so no denorm/inf edge cases
                    nc.vector.reciprocal_approx_fast(out=rstd[:], in_=std[:])
                    nc.vector.tensor_tensor(out=scalef[:], in0=gb_sb[:, 0:2], in1=rstd[:], op=OP.mult)
                    nc.vector.tensor_tensor(out=msc[:], in0=mean[:], in1=scalef[:], op=OP.mult)
                    nc.vector.tensor_tensor(out=shift[:], in0=gb_sb[:, 2:4], in1=msc[:], op=OP.subtract)
                    for g in range(NG):
                        nc.vector.tensor_scalar_mul(out=diag_sb[:, g, :], in0=ident_sb[:],
                                                    scalar1=scalef[:, g:g + 1])

                # =================== phase 2: recurrence ===================
                for t in range(T if ph2 else 0):
                    hp_prev = h_pads[(t - 1) % 2]
                    hp = h_pads[t % 2]
                    hv = hp[:].rearrange("p (r c) -> p r c", c=WP)

                    y0t = iop.tile([128, NG, PX], BF16, tag="y0t", name="y0t")
                    nc.sync.dma_start(out=y0t[:], in_=y0_dram[t])

                    if_t = workp.tile([128, PX], BF16, tag="if_t", name="if_t")
                    o_t = workp.tile([64, PX], BF16, tag="o_t", name="o_t")
                    f0_t = workp.tile([64, PX], BF16, tag="f0_t", name="f0_t")
                    g0_t = workp.tile([64, PX], BF16, tag="g0_t", name="g0_t")
                    th_t = workp.tile([64, PX], BF16, tag="th_t", name="th_t")

                    # Uniform 8-row chunks; first/last need the halo.
                    # All diag(scale)@y0 matmuls are issued first: they depend
                    # only on y0t (not on h_{t-1} or the halo), so PE has
                    # accumulator-seeding matmuls to chew on while the
                    # previous step's halo collective completes.  (4-row
                    # boundary chunks were tried: the extra chunk's issue cost
                    # outweighed the shorter halo-dependent tail.)
                    if exchange and NCH > 2:
                        chunks = [(j * CHR, CHR) for j in range(NCH)]
                        bnd_idx = [0, len(chunks) - 1]
                        int_idx = list(range(1, len(chunks) - 1))
                        ps_t = {}
                        for ci, (r0, nr) in enumerate(chunks):
                            for g in range(NG):
                                ps_t[(ci, g)] = psump.tile(
                                    [128, nr * W], F32, tag="ps", name="ps")[:]
                                nc.tensor.matmul(ps_t[(ci, g)], diag_sb[:, g, :],
                                                 y0t[:, g, r0 * W:(r0 + nr) * W],
                                                 start=True, stop=(t == 0))
                    else:
                        chunks = [(j * CHR, CHR) for j in range(NCH)]
                        bnd_idx = list(range(len(chunks)))
                        int_idx = []
                        ps_t = {}

                    def do_taps(ci):
                        r0, nr = chunks[ci]
                        if not (exchange and NCH > 2):
                            # v1 path: seed per chunk (sequential PSUM reuse)
                            for g in range(NG):
                                ps = psump.tile([128, nr * W], F32, tag="ps", name="ps")[:]
                                ps_t[(ci, g)] = ps
                                nc.tensor.matmul(ps, diag_sb[:, g, :],
                                                 y0t[:, g, r0 * W:(r0 + nr) * W],
                                                 start=True, stop=(t == 0))
                        if t > 0:
                            for g in range(NG):
                                conv_mms(ps_t[(ci, g)], hp_prev, 1, g, r0, nr,
                                         first_start=False)

                    def do_gates(ci):
                        r0, nr = chunks[ci]
                        sl = slice(r0 * W, (r0 + nr) * W)
                        for g in range(NG):
                            ps = ps_t[(ci, g)]
                            if g == 0:
                                nc.scalar.activation(out=if_t[:, sl], in_=ps,
                                                     func=AF.Sigmoid, bias=shift[:, 0:1])
                                # f lives on partitions 64..127; move to 0..63 (DMA
                                # is the only engine allowed to change partitions)
                                nc.scalar.dma_start(out=f0_t[:, sl], in_=if_t[64:128, sl])
                            else:
                                nc.scalar.activation(out=o_t[:, sl], in_=ps[0:64, :],
                                                     func=AF.Sigmoid, bias=shift[0:64, 1:2])
                                ghi = workp.tile([128, CPX], BF16, tag="ghi", name="ghi")
                                nc.scalar.activation(out=ghi[64:128, 0:nr * W], in_=ps[64:128, :],
                                                     func=AF.Tanh, bias=shift[64:128, 1:2])
                                nc.scalar.dma_start(out=g0_t[:, sl], in_=ghi[64:128, 0:nr * W])

                    def do_elem(ci, rs=0, rc=None, rowshift=True):
                        # elementwise state update for rows rs..rs+rc of chunk
                        # ci (relative); rowshift=True also emits the chunk's
                        # full-extent row-shift copy (must follow all parts)
                        r0, nr = chunks[ci]
                        if rc is None:
                            rc = nr
                        a0 = r0 + rs
                        sl = slice(a0 * W, (a0 + rc) * W)
                        i_ap = if_t[0:64, sl]
                        f_ap = f0_t[:, sl]
                        o_ap = o_t[:, sl]
                        g_ap = g0_t[:, sl]
                        c_ap = c_sb[:, sl]
                        if t == 0:
                            nc.vector.tensor_tensor(out=c_ap, in0=i_ap, in1=g_ap, op=OP.mult)
                        else:
                            ig = workp.tile([64, CPX], F32, tag="ig", name="ig")
                            ig_ap = ig[0:64, 0:rc * W]
                            nc.vector.tensor_tensor(out=ig_ap, in0=i_ap, in1=g_ap, op=OP.mult)
                            nc.vector.tensor_tensor(out=c_ap, in0=f_ap, in1=c_ap, op=OP.mult)
                            nc.vector.tensor_tensor(out=c_ap, in0=c_ap, in1=ig_ap, op=OP.add)
                        nc.scalar.activation(out=th_t[:, sl], in_=c_ap, func=AF.Tanh)
                        h_dst = hv[0:64, a0 + 1:a0 + 1 + rc, 1:W + 1]
                        o3 = o_t[:, sl].rearrange("p (r c) -> p r c", c=W)
                        t3 = th_t[:, sl].rearrange("p (r c) -> p r c", c=W)
                        nc.vector.tensor_tensor(out=h_dst, in0=o3, in1=t3, op=OP.mult)
                        if rowshift:
                            # row-shift(+2) copy of the chunk's rows into
                            # partitions 64..127
                            d0 = max(0, r0 - 1) * WP
                            d1 = (r0 + nr - 1) * WP
                            nc.sync.dma_start(out=hp[64:128, d0:d1],
                                              in_=hp[0:64, d0 + 2 * WP:d1 + 2 * WP])
                    # Matmul order is decoupled from act/elementwise order:
                    # interior-chunk taps depend only on locally written h of
                    # step t-1, so they run FIRST and keep PE busy while the
                    # previous step's halo collective lands; boundary-chunk
                    # taps (which need the halo) follow.  The act/elementwise
                    # pass then finishes the boundary chunks first so their h
                    # rows feed this step's exchange as early as possible,
                    # with the interior act/elementwise emitted after it.
                    for ci in int_idx + bnd_idx:
                        do_taps(ci)
                    split = exchange and not skip_cc and t < T - 1 and NCH > 2
                    for ci in bnd_idx:
                        do_gates(ci)
                    if split:
                        # only the single h row each send needs is computed
                        # before the exchange: row 0 of chunk 0 (hv row 1) and
                        # the last row of the last chunk (hv row HL) -- the
                        # sends then fire ~a full-chunk elementwise earlier
                        do_elem(0, 0, 1, rowshift=False)
                        lc = chunks[NCH - 1]
                        do_elem(NCH - 1, lc[1] - 1, 1, rowshift=False)

                    # ---- halo exchange with the pair partner ----
                    if exchange and not skip_cc and t < T - 1:
                        cin, cout_ = cch_in[t % 2], cch_out[t % 2]
                        # send my first own row (slot A) and last own row (slot B)
                        nc.sync.dma_start(out=cin[:, 0:W], in_=hv[0:64, 1, 1:W + 1])
                        nc.sync.dma_start(out=cin[:, W:2 * W], in_=hv[0:64, HL, 1:W + 1])
                        nc.gpsimd.collective_compute(
                            "AllGather", OP.bypass, replica_groups=pair_groups,
                            ins=[cin[:]], outs=[cout_[:]],
                        )
                        ccs = ccp.tile([128, 2 * W], BF16, tag="ccs", name="ccs")
                        nc.sync.dma_start(out=ccs[:], in_=cout_[:])
                        # partner's first row (rank1 slot A) moved to partitions 0..63
                        cclo = ccp.tile([64, W], BF16, tag="cclo", name="cclo")
                        nc.sync.dma_start(out=cclo[:], in_=ccs[64:128, 0:W])
                        # masked halo writes run on the (otherwise idle) Pool
                        # engine: they wait on the collective, and on DVE they
                        # would block the whole in-order DVE queue -- stalling
                        # the interior chunks' elementwise emitted after them
                        # top halo row 0 <- rank0's last row (masked: 0 on rank0)
                        nc.gpsimd.tensor_scalar_mul(
                            out=hv[0:64, 0, 1:W + 1],
                            in0=ccs[0:64, W:2 * W].rearrange("p (r c) -> p r c", c=W),
                            scalar1=hm_sb[0:64, 0:1])
                        # bottom halo row HL+1 <- rank1's first row (masked: 0 on rank1)
                        nc.gpsimd.tensor_scalar_mul(
                            out=hv[0:64, RP - 1, 1:W + 1],
                            in0=cclo[:].rearrange("p (r c) -> p r c", c=W),
                            scalar1=hm_sb[0:64, 1:2])
                        # same bottom-halo data into the row-shift image (p64 row HL-1)
                        nc.gpsimd.tensor_scalar_mul(
                            out=hp[64:128, (HL - 1) * WP + 1:(HL - 1) * WP + 1 + W],
                            in0=ccs[64:128, 0:W],
                            scalar1=hm_sb[64:128, 1:2])

                    if split:
                        do_elem(0, 1, chunks[0][1] - 1)
                        do_elem(NCH - 1, 0, chunks[NCH - 1][1] - 1)
                    else:
                        for ci in bnd_idx:
                            do_elem(ci)
                    for ci in int_idx:
                        do_gates(ci)
                        do_elem(ci)

                    # ---- write h_t to output ----
                    ov = out_ext[t].rearrange("p (r c) -> p r c", c=W)
                    nc.gpsimd.dma_start(out=ov, in_=hv[0:64, 1:HL + 1, 1:W + 1])
            for _rep in range(repeat):
                _body()

    nc.finalize()
    return nc


def pack_weights(Wx, Wh):
    """Pack [256,64,3,3] OIHW conv weights into the [128, 24, 128] lhsT tensor."""
    w = np.zeros((128, 24, 128), np.float32)
    for conv, Wc in ((0, Wx), (1, Wh)):
        for g in range(NG):
            for kw in range(3):
                # pair slot: rows 0:64 tap (kh=0), rows 64:128 tap (kh=2)
                s = _slot(conv, g, "pair", kw)
                w[0:64, s, :] = Wc[128 * g:128 * (g + 1), :, 0, kw].T
                w[64:128, s, :] = Wc[128 * g:128 * (g + 1), :, 2, kw].T
                # single slot: rows 0:64 tap (kh=1)
                s = _slot(conv, g, "single", kw)
                w[0:64, s, :] = Wc[128 * g:128 * (g + 1), :, 1, kw].T
    return w.astype(ml_dtypes.bfloat16)


def make_in_maps(x, Wx, gamma, beta, Wh, HL, exchange, n_cores):
    """Build per-core input dicts. Core c handles batch n = c//2; with
    exchange, odd/even cores take the bottom/top H-half."""
    x = np.asarray(x, np.float32)
    w = pack_weights(np.asarray(Wx, np.float32), np.asarray(Wh, np.float32))
    gamma = np.asarray(gamma, np.float32)
    beta = np.asarray(beta, np.float32)
    gb = np.stack([gamma[0:128], gamma[128:256],
                   beta[0:128], beta[128:256]], axis=1).astype(np.float32)
    ident = np.eye(128, dtype=ml_dtypes.bfloat16)
    T, N, _, H, W = x.shape
    xpad = np.zeros((T, N, CIN, H + 2, W + 2), np.float32)
    xpad[:, :, :, 1:H + 1, 1:W + 1] = x
    xpad = xpad.astype(ml_dtypes.bfloat16)
    in_maps = []
    for c in range(n_cores):
        n, s = c // 2, c % 2
        r0 = s * HL if exchange else 0
        xc = np.ascontiguousarray(xpad[:, n, :, r0:r0 + HL + 2, :])
        if exchange:
            hm = np.array([[float(s == 1), float(s == 0)]], np.float32)
        else:
            hm = np.zeros((1, 2), np.float32)
        hm = np.broadcast_to(hm, (128, 2)).copy()
        in_maps.append({"x": xc, "w": w, "gb": gb, "ident": ident, "hm": hm})
    return in_maps


_last_results = None


def assemble(per_core_results, T, N, H, W, HL, exchange):
    """Gather per-core "out" buffers into the full hs [T, N, 64, H, W]."""
    hs = np.empty((T, N, CIN, H, W), np.float32)
    for n in range(N):
        if exchange:
            for s in range(2):
                o = np.asarray(per_core_results[2 * n + s]["out"]).astype(np.float32)
                hs[:, n, :, s * HL:(s + 1) * HL, :] = o.reshape(T, CIN, HL, W)
        else:
            o = np.asarray(per_core_results[2 * n]["out"]).astype(np.float32)
            hs[:, n] = o.reshape(T, CIN, H, W)
    return hs


def kernel(x, Wx, bx, gamma, beta, Wh, exchange=True):
    """Full-input entry point: returns hs [T, N, 64, H, W] float32."""
    global _last_results
    T, N, _, H, W = np.asarray(x).shape
    n_cores = 2 * N
    HL = H // 2 if exchange else H
    nc = build(T=T, HL=HL, W=W, n_cores=n_cores, exchange=exchange)
    in_maps = make_in_maps(x, Wx, gamma, beta, Wh, HL, exchange, n_cores)
    import time as _time
    _t0 = _time.monotonic()
    res = run_bass_kernel_spmd(nc, in_maps, list(range(n_cores)))
    globals()["_last_spmd_s"] = _time.monotonic() - _t0
    _last_results = res
    return assemble(res.results, T, N, H, W, HL, exchange)



# revision 47
# speedup vs baseline: 1.0755x; 1.0755x over previous
"""ConvLSTM Trainium2 kernel (8 NeuronCores, SPMD).

Reference computation:
  phase 1: y = conv3x3(x, Wx) + bx  over all T*N frames,
           BatchNorm(train, biased var) over (T*N, H, W) per channel,
           y = y_hat * gamma + beta
  phase 2: per t: tmp = conv3x3(h, Wh) + y[t]; i,f,o = sigmoid, g = tanh
           c = f*c + i*g ; h = o*tanh(c)
  output hs: [T, N, 64, H, W]

Key algebra used here:
  - the conv bias bx cancels exactly inside BatchNorm (mean removes it),
    so bx is never used.
  - BN is folded to y_norm = y0*scale_c + shift_c with
    scale = gamma*rsqrt(var+eps), shift = beta - mean*scale, where y0 is
    the raw (bias-free) conv output.  scale is applied by an extra
    "diag(scale)" matmul accumulated into the same PSUM group as the h2h
    conv; shift is folded into the gate activation's per-partition bias.

Distribution (default, exchange=True): core c handles batch n = c//2 and
H-half s = c%2 (top/bottom 32 rows).  Phase 1 needs no communication (x
arrives host-pre-padded with halo rows); the recurrence swaps one halo
row of h per step with the pair partner via a 2-rank AllGather, masked
per-core so global edges stay zero.  BN statistics are summed across all
8 cores with one tiny AllReduce.  exchange=False falls back to pairs
duplicating the full-H recurrence (no per-step collectives).

Conv-as-matmul: 3x3 taps become 6 matmuls per 128-out-channel group per
512-pixel chunk: 3 "pair" matmuls (kh=0 and kh=2 packed along K=128 via a
row-shifted copy of the input living in SBUF partitions 64..127) plus 3
"single" matmuls (kh=1, K=64).
"""

import os
import numpy as np
import ml_dtypes

import concourse.bass as bass
import concourse.bacc as bacc
import concourse.mybir as mybir
from concourse.tile import TileContext
from concourse.bass_utils import run_bass_kernel_spmd

BF16 = mybir.dt.bfloat16
F32 = mybir.dt.float32
AF = mybir.ActivationFunctionType
OP = mybir.AluOpType

BN_EPS = 1e-5
CIN = 64   # conv input channels (also hidden size)
NG = 2     # output-channel groups of 128 (4*64 = 256 = 2*128)


def _slot(conv, g, kind, kw):
    """Weight slot index in the packed [128, 24, 128] lhsT tensor."""
    return conv * 12 + g * 6 + (kw if kind == "pair" else 3 + kw)


def build(T=16, HL=64, W=64, n_cores=8, exchange=False, pool_bufs=2, repeat=1,
          skip_cc=False, ph1=True, ph2=True):
    # skip_cc=True omits the per-step halo AllGather (WRONG results;
    # timing-only, to isolate collective cost).
    # HL = rows of H owned by this core.  exchange=False: every core holds
    # the full H (pairs duplicate phase-2 work).  exchange=True: pairs
    # split H in half and swap one halo row per step via a 2-rank
    # AllGather.
    RP, WP = HL + 2, W + 2         # local padded rows / cols
    L = RP * WP                    # flat padded length
    CHR = 8                        # rows per chunk
    assert HL % CHR == 0
    NCH = HL // CHR                # chunks per local frame
    CPX = CHR * W                  # pixels per chunk (<= 512)
    assert CPX <= 512
    PX = HL * W
    NCOL = T * NCH                 # stat columns per group

    nc = bacc.Bacc(num_devices=n_cores)

    x_ext = nc.declare_dram_parameter("x", [T, CIN, RP, WP], BF16, isOutput=False)
    w_ext = nc.declare_dram_parameter("w", [128, 24, 128], BF16, isOutput=False)
    gb_ext = nc.declare_dram_parameter("gb", [128, 4], F32, isOutput=False)
    id_ext = nc.declare_dram_parameter("ident", [128, 128], BF16, isOutput=False)
    hm_ext = nc.declare_dram_parameter("hm", [128, 2], F32, isOutput=False)
    out_ext = nc.declare_dram_parameter("out", [T, CIN, PX], BF16, isOutput=True)

    y0_dram = nc.dram_tensor("y0_buf", [T, 128, NG, PX], BF16)
    cc_in = nc.dram_tensor("cc_in", [128, 4], F32)
    cc_out = nc.dram_tensor("cc_out", [128, 4], F32, addr_space="Shared")
    if exchange:
        # double-buffered halo bounce buffers (one pair per step parity)
        cch_in = [nc.dram_tensor(f"cch_in{i}", [64, 2 * W], BF16) for i in range(2)]
        cch_out = [nc.dram_tensor(f"cch_out{i}", [128, 2 * W], BF16) for i in range(2)]
        pair_groups = [[2 * i, 2 * i + 1] for i in range(n_cores // 2)]

    def conv_mms(ps, pad_tile, conv, g, r0, nr, first_start):
        """6 matmuls accumulating conv tap contributions for rows r0..r0+nr."""
        v = pad_tile[:].rearrange("p (r c) -> p r c", c=WP)
        taps = [("pair", 0), ("pair", 1), ("pair", 2),
                ("single", 0), ("single", 1), ("single", 2)]
        for idx, (kind, kw) in enumerate(taps):
            s = _slot(conv, g, kind, kw)
            if kind == "pair":
                lhsT = w_sb[:, s, :]
                rhs = v[:, r0:r0 + nr, kw:kw + W]
            else:
                lhsT = w_sb[0:64, s, :]
                rhs = v[0:64, r0 + 1:r0 + 1 + nr, kw:kw + W]
            nc.tensor.matmul(
                ps, lhsT, rhs,
                start=(first_start and idx == 0),
                stop=(idx == len(taps) - 1),
            )

    with TileContext(nc) as tc:
        with (
            tc.tile_pool(name="const", bufs=1) as constp,
            tc.tile_pool(name="state", bufs=1) as statep,
            tc.tile_pool(name="io", bufs=pool_bufs) as iop,
            tc.tile_pool(name="cc", bufs=pool_bufs) as ccp,
            tc.tile_pool(name="work", bufs=pool_bufs) as workp,
            tc.tile_pool(name="psum", bufs=8, space="PSUM") as psump,
        ):
            # ---- constants ----
            w_sb = constp.tile([128, 24, 128], BF16, tag="w_sb", name="w_sb")
            ident_sb = constp.tile([128, 128], BF16, tag="ident_sb", name="ident_sb")
            gb_sb = constp.tile([128, 4], F32, tag="gb_sb", name="gb_sb")
            hm_sb = constp.tile([128, 2], F32, tag="hm_sb", name="hm_sb")
            nc.sync.dma_start(out=w_sb[:], in_=w_ext[:])
            nc.sync.dma_start(out=ident_sb[:], in_=id_ext[:])
            nc.sync.dma_start(out=gb_sb[:], in_=gb_ext[:])
            nc.sync.dma_start(out=hm_sb[:], in_=hm_ext[:])

            # ---- persistent state ----
            x_pads = [statep.tile([128, L], BF16, tag=f"x_pad{i}", name=f"x_pad{i}") for i in range(2)]
            h_pads = [statep.tile([128, L], BF16, tag=f"h_pad{i}", name=f"h_pad{i}") for i in range(2)]
            c_sb = statep.tile([64, PX], F32, tag="c_sb", name="c_sb")
            sum_cols = statep.tile([128, NG, NCOL], F32, tag="sum_cols", name="sum_cols")
            sq_cols = statep.tile([128, NG, NCOL], F32, tag="sq_cols", name="sq_cols")
            s1 = statep.tile([128, NG], F32, tag="s1", name="s1")
            s2 = statep.tile([128, NG], F32, tag="s2", name="s2")
            mean = statep.tile([128, NG], F32, tag="mean", name="mean")
            ex2 = statep.tile([128, NG], F32, tag="ex2", name="ex2")
            m2 = statep.tile([128, NG], F32, tag="m2", name="m2")
            var = statep.tile([128, NG], F32, tag="var", name="var")
            std = statep.tile([128, NG], F32, tag="std", name="std")
            rstd = statep.tile([128, NG], F32, tag="rstd", name="rstd")
            scalef = statep.tile([128, NG], F32, tag="scalef", name="scalef")
            msc = statep.tile([128, NG], F32, tag="msc", name="msc")
            shift = statep.tile([128, NG], F32, tag="shift", name="shift")
            red = statep.tile([128, 4], F32, tag="red", name="red")
            diag_sb = statep.tile([128, NG, 128], BF16, tag="diag_sb", name="diag_sb")

            # repeat>1 re-runs the whole computation back-to-back in one
            # launch -- measurement-only (exec time = slope vs repeat);
            # outputs are still those of the last rep's recurrence.
            def _body():
                # Zero h_pads on DVE (not gpsimd.memset): the first matmuls
                # reading h_pad already wait on DVE, so this adds no extra
                # sync-wait (MMs allow at most 3).
                zero_l = nc.const_aps.tensor(0.0, (128, L), F32)
                for tile_ in h_pads:
                    nc.vector.tensor_copy(out=tile_[:], in_=zero_l)

                # =================== phase 1: x2h conv + stats ===================
                for t in range(T if ph1 else 0):
                    xp = x_pads[t % 2]
                    xv = xp[:].rearrange("p (r c) -> p r c", c=WP)
                    # x arrives pre-padded [CIN, RP, WP]; base copy fills the
                    # whole tile (borders included), so no on-device memset.
                    nc.sync.dma_start(out=xv[0:64, :, :], in_=x_ext[t])
                    # row-shift(+2) copy: p64 row r = padded row r+2
                    nc.sync.dma_start(out=xv[64:128, 0:HL, :], in_=x_ext[t, :, 2:HL + 2, :])

                    y0s = iop.tile([128, NG, PX], BF16, tag="y0t", name="y0t")
                    for g in range(NG):
                        for j in range(NCH):
                            ps = psump.tile([128, CPX], F32, tag="ps", name="ps")
                            conv_mms(ps[:], xp, 0, g, j * CHR, CHR, first_start=True)
                            col = t * NCH + j
                            # psum -> bf16 y0 slice, plus channel sum (accum_out)
                            nc.vector.tensor_scalar(
                                out=y0s[:, g, j * CPX:(j + 1) * CPX],
                                in0=ps[:], scalar1=1.0, scalar2=0.0,
                                op0=OP.mult, op1=OP.add,
                                accum_out=sum_cols[:, g, col:col + 1],
                            )
                            # channel sum of squares
                            scr = workp.tile([128, CPX], BF16, tag="sqscr", name="sqscr")
                            nc.scalar.activation(
                                out=scr[:], in_=ps[:], func=AF.Square,
                                accum_out=sq_cols[:, g, col:col + 1],
                            )
                    nc.sync.dma_start(out=y0_dram[t], in_=y0s[:])

                # =================== BN stats -> scale/shift ===================
                if not ph1:
                    # timing-only stub (ph2-only build): unit scale, zero shift
                    nc.vector.tensor_scalar_mul(out=shift[:], in0=gb_sb[:, 2:4],
                                                scalar1=0.0)
                    for g in range(NG):
                        nc.vector.tensor_copy(out=diag_sb[:, g, :], in_=ident_sb[:])
                else:
                    nc.vector.tensor_reduce(out=s1[:], in_=sum_cols[:],
                                            axis=mybir.AxisListType.X, op=OP.add)
                    nc.vector.tensor_reduce(out=s2[:], in_=sq_cols[:],
                                            axis=mybir.AxisListType.X, op=OP.add)
                    nc.sync.dma_start(out=cc_in[:, 0:2], in_=s1[:])
                    nc.sync.dma_start(out=cc_in[:, 2:4], in_=s2[:])
                    nc.gpsimd.collective_compute(
                        "AllReduce", OP.add,
                        replica_groups=[list(range(n_cores))],
                        ins=[cc_in[:]], outs=[cc_out[:]],
                    )
                    nc.sync.dma_start(out=red[:], in_=cc_out[:])
                    inv = 1.0 / float(n_cores * T * HL * W)
                    nc.vector.tensor_scalar_mul(out=mean[:], in0=red[:, 0:2], scalar1=inv)
                    nc.vector.tensor_scalar_mul(out=ex2[:], in0=red[:, 2:4], scalar1=inv)
                    nc.vector.tensor_tensor(out=m2[:], in0=mean[:], in1=mean[:], op=OP.mult)
                    # var+eps = (ex2 + eps) - mean^2, fused in one op
                    nc.vector.scalar_tensor_tensor(out=var[:], in0=ex2[:], scalar=BN_EPS,
                                                   in1=m2[:], op0=OP.add, op1=OP.subtract)
                    nc.scalar.activation(out=std[:], in_=var[:], func=AF.Sqrt)
                    # ~18-bit 1/x, ~5x faster than InstReciprocal; var+eps is
                    # O(1) here so no denorm/inf edge cases
                    nc.vector.reciprocal_approx_fast(out=rstd[:], in_=std[:])
                    nc.vector.tensor_tensor(out=scalef[:], in0=gb_sb[:, 0:2], in1=rstd[:], op=OP.mult)
                    nc.vector.tensor_tensor(out=msc[:], in0=mean[:], in1=scalef[:], op=OP.mult)
                    nc.vector.tensor_tensor(out=shift[:], in0=gb_sb[:, 2:4], in1=msc[:], op=OP.subtract)
                    for g in range(NG):
                        nc.vector.tensor_scalar_mul(out=diag_sb[:, g, :], in0=ident_sb[:],
                                                    scalar1=scalef[:, g:g + 1])

                # =================== phase 2: recurrence ===================
                for t in range(T if ph2 else 0):
                    hp_prev = h_pads[(t - 1) % 2]
                    hp = h_pads[t % 2]
                    hv = hp[:].rearrange("p (r c) -> p r c", c=WP)

                    y0t = iop.tile([128, NG, PX], BF16, tag="y0t", name="y0t")
                    nc.sync.dma_start(out=y0t[:], in_=y0_dram[t])

                    if_t = workp.tile([128, PX], BF16, tag="if_t", name="if_t")
                    o_t = workp.tile([64, PX], BF16, tag="o_t", name="o_t")
                    f0_t = workp.tile([64, PX], BF16, tag="f0_t", name="f0_t")
                    g0_t = workp.tile([64, PX], BF16, tag="g0_t", name="g0_t")
                    th_t = workp.tile([64, PX], BF16, tag="th_t", name="th_t")

                    # Uniform 8-row chunks; first/last need the halo.
                    # All diag(scale)@y0 matmuls are issued first: they depend
                    # only on y0t (not on h_{t-1} or the halo), so PE has
                    # accumulator-seeding matmuls to chew on while the
                    # previous step's halo collective completes.  (4-row
                    # boundary chunks were tried: the extra chunk's issue cost
                    # outweighed the shorter halo-dependent tail.)
                    if exchange and NCH > 2:
                        chunks = [(j * CHR, CHR) for j in range(NCH)]
                        bnd_idx = [0, len(chunks) - 1]
                        int_idx = list(range(1, len(chunks) - 1))
                        ps_t = {}
                        for ci, (r0, nr) in enumerate(chunks):
                            for g in range(NG):
                                ps_t[(ci, g)] = psump.tile(
                                    [128, nr * W], F32, tag="ps", name="ps")[:]
                                nc.tensor.matmul(ps_t[(ci, g)], diag_sb[:, g, :],
                                                 y0t[:, g, r0 * W:(r0 + nr) * W],
                                                 start=True, stop=(t == 0))
                    else:
                        chunks = [(j * CHR, CHR) for j in range(NCH)]
                        bnd_idx = list(range(len(chunks)))
                        int_idx = []
                        ps_t = {}

                    def do_taps(ci):
                        r0, nr = chunks[ci]
                        if not (exchange and NCH > 2):
                            # v1 path: seed per chunk (sequential PSUM reuse)
                            for g in range(NG):
                                ps = psump.tile([128, nr * W], F32, tag="ps", name="ps")[:]
                                ps_t[(ci, g)] = ps
                                nc.tensor.matmul(ps, diag_sb[:, g, :],
                                                 y0t[:, g, r0 * W:(r0 + nr) * W],
                                                 start=True, stop=(t == 0))
                        if t > 0:
                            for g in range(NG):
                                conv_mms(ps_t[(ci, g)], hp_prev, 1, g, r0, nr,
                                         first_start=False)

                    def do_gates(ci):
                        r0, nr = chunks[ci]
                        sl = slice(r0 * W, (r0 + nr) * W)
                        for g in range(NG):
                            ps = ps_t[(ci, g)]
                            if g == 0:
                                nc.scalar.activation(out=if_t[:, sl], in_=ps,
                                                     func=AF.Sigmoid, bias=shift[:, 0:1])
                                # f lives on partitions 64..127; move to 0..63 (DMA
                                # is the only engine allowed to change partitions)
                                nc.scalar.dma_start(out=f0_t[:, sl], in_=if_t[64:128, sl])
                            else:
                                nc.scalar.activation(out=o_t[:, sl], in_=ps[0:64, :],
                                                     func=AF.Sigmoid, bias=shift[0:64, 1:2])
                                ghi = workp.tile([128, CPX], BF16, tag="ghi", name="ghi")
                                nc.scalar.activation(out=ghi[64:128, 0:nr * W], in_=ps[64:128, :],
                                                     func=AF.Tanh, bias=shift[64:128, 1:2])
                                nc.sync.dma_start(out=g0_t[:, sl], in_=ghi[64:128, 0:nr * W])

                    def do_elem(ci, rs=0, rc=None, rowshift=True):
                        # elementwise state update for rows rs..rs+rc of chunk
                        # ci (relative); rowshift=True also emits the chunk's
                        # full-extent row-shift copy (must follow all parts)
                        r0, nr = chunks[ci]
                        if rc is None:
                            rc = nr
                        a0 = r0 + rs
                        sl = slice(a0 * W, (a0 + rc) * W)
                        i_ap = if_t[0:64, sl]
                        f_ap = f0_t[:, sl]
                        o_ap = o_t[:, sl]
                        g_ap = g0_t[:, sl]
                        c_ap = c_sb[:, sl]
                        if t == 0:
                            nc.vector.tensor_tensor(out=c_ap, in0=i_ap, in1=g_ap, op=OP.mult)
                        else:
                            ig = workp.tile([64, CPX], F32, tag="ig", name="ig")
                            ig_ap = ig[0:64, 0:rc * W]
                            nc.vector.tensor_tensor(out=ig_ap, in0=i_ap, in1=g_ap, op=OP.mult)
                            nc.vector.tensor_tensor(out=c_ap, in0=f_ap, in1=c_ap, op=OP.mult)
                            nc.vector.tensor_tensor(out=c_ap, in0=c_ap, in1=ig_ap, op=OP.add)
                        nc.scalar.activation(out=th_t[:, sl], in_=c_ap, func=AF.Tanh)
                        h_dst = hv[0:64, a0 + 1:a0 + 1 + rc, 1:W + 1]
                        o3 = o_t[:, sl].rearrange("p (r c) -> p r c", c=W)
                        t3 = th_t[:, sl].rearrange("p (r c) -> p r c", c=W)
                        nc.vector.tensor_tensor(out=h_dst, in0=o3, in1=t3, op=OP.mult)
                        if rowshift:
                            # row-shift(+2) copy of the chunk's rows into
                            # partitions 64..127
                            d0 = max(0, r0 - 1) * WP
                            d1 = (r0 + nr - 1) * WP
                            nc.sync.dma_start(out=hp[64:128, d0:d1],
                                              in_=hp[0:64, d0 + 2 * WP:d1 + 2 * WP])
                    # Matmul order is decoupled from act/elementwise order:
                    # interior-chunk taps depend only on locally written h of
                    # step t-1, so they run FIRST and keep PE busy while the
                    # previous step's halo collective lands; boundary-chunk
                    # taps (which need the halo) follow.  The act/elementwise
                    # pass then finishes the boundary chunks first so their h
                    # rows feed this step's exchange as early as possible,
                    # with the interior act/elementwise emitted after it.
                    for ci in int_idx + bnd_idx:
                        do_taps(ci)
                    split = exchange and not skip_cc and t < T - 1 and NCH > 2
                    for ci in bnd_idx:
                        do_gates(ci)
                    if split:
                        # only the single h row each send needs is computed
                        # before the exchange: row 0 of chunk 0 (hv row 1) and
                        # the last row of the last chunk (hv row HL) -- the
                        # sends then fire ~a full-chunk elementwise earlier
                        do_elem(0, 0, 1, rowshift=False)
                        lc = chunks[NCH - 1]
                        do_elem(NCH - 1, lc[1] - 1, 1, rowshift=False)

                    # ---- halo exchange with the pair partner ----
                    if exchange and not skip_cc and t < T - 1:
                        cin, cout_ = cch_in[t % 2], cch_out[t % 2]
                        # send my first own row (slot A) and last own row (slot B)
                        nc.scalar.dma_start(out=cin[:, 0:W], in_=hv[0:64, 1, 1:W + 1])
                        nc.scalar.dma_start(out=cin[:, W:2 * W], in_=hv[0:64, HL, 1:W + 1])
                        nc.gpsimd.collective_compute(
                            "AllGather", OP.bypass, replica_groups=pair_groups,
                            ins=[cin[:]], outs=[cout_[:]],
                        )
                        ccs = ccp.tile([128, 2 * W], BF16, tag="ccs", name="ccs")
                        nc.sync.dma_start(out=ccs[:], in_=cout_[:])
                        # partner's first row (rank1 slot A) moved to partitions 0..63
                        cclo = ccp.tile([64, W], BF16, tag="cclo", name="cclo")
                        nc.sync.dma_start(out=cclo[:], in_=ccs[64:128, 0:W])
                        # masked halo writes run on the (otherwise idle) Pool
                        # engine: they wait on the collective, and on DVE they
                        # would block the whole in-order DVE queue -- stalling
                        # the interior chunks' elementwise emitted after them
                        # top halo row 0 <- rank0's last row (masked: 0 on rank0)
                        nc.gpsimd.tensor_scalar_mul(
                            out=hv[0:64, 0, 1:W + 1],
                            in0=ccs[0:64, W:2 * W].rearrange("p (r c) -> p r c", c=W),
                            scalar1=hm_sb[0:64, 0:1])
                        # bottom halo row HL+1 <- rank1's first row (masked: 0 on rank1)
                        nc.gpsimd.tensor_scalar_mul(
                            out=hv[0:64, RP - 1, 1:W + 1],
                            in0=cclo[:].rearrange("p (r c) -> p r c", c=W),
                            scalar1=hm_sb[0:64, 1:2])
                        # same bottom-halo data into the row-shift image (p64 row HL-1)
                        nc.gpsimd.tensor_scalar_mul(
                            out=hp[64:128, (HL - 1) * WP + 1:(HL - 1) * WP + 1 + W],
                            in0=ccs[64:128, 0:W],
                            scalar1=hm_sb[64:128, 1:2])

                    if split:
                        do_elem(0, 1, chunks[0][1] - 1)
                        do_elem(NCH - 1, 0, chunks[NCH - 1][1] - 1)
                    else:
                        for ci in bnd_idx:
                            do_elem(ci)
                    for ci in int_idx:
                        do_gates(ci)
                        do_elem(ci)

                    # ---- write h_t to output ----
                    ov = out_ext[t].rearrange("p (r c) -> p r c", c=W)
                    nc.gpsimd.dma_start(out=ov, in_=hv[0:64, 1:HL + 1, 1:W + 1])
            for _rep in range(repeat):
                _body()

    nc.finalize()
    return nc


def pack_weights(Wx, Wh):
    """Pack [256,64,3,3] OIHW conv weights into the [128, 24, 128] lhsT tensor."""
    w = np.zeros((128, 24, 128), np.float32)
    for conv, Wc in ((0, Wx), (1, Wh)):
        for g in range(NG):
            for kw in range(3):
                # pair slot: rows 0:64 tap (kh=0), rows 64:128 tap (kh=2)
                s = _slot(conv, g, "pair", kw)
                w[0:64, s, :] = Wc[128 * g:128 * (g + 1), :, 0, kw].T
                w[64:128, s, :] = Wc[128 * g:128 * (g + 1), :, 2, kw].T
                # single slot: rows 0:64 tap (kh=1)
                s = _slot(conv, g, "single", kw)
                w[0:64, s, :] = Wc[128 * g:128 * (g + 1), :, 1, kw].T
    return w.astype(ml_dtypes.bfloat16)


def make_in_maps(x, Wx, gamma, beta, Wh, HL, exchange, n_cores):
    """Build per-core input dicts. Core c handles batch n = c//2; with
    exchange, odd/even cores take the bottom/top H-half."""
    x = np.asarray(x, np.float32)
    w = pack_weights(np.asarray(Wx, np.float32), np.asarray(Wh, np.float32))
    gamma = np.asarray(gamma, np.float32)
    beta = np.asarray(beta, np.float32)
    gb = np.stack([gamma[0:128], gamma[128:256],
                   beta[0:128], beta[128:256]], axis=1).astype(np.float32)
    ident = np.eye(128, dtype=ml_dtypes.bfloat16)
    T, N, _, H, W = x.shape
    xpad = np.zeros((T, N, CIN, H + 2, W + 2), np.float32)
    xpad[:, :, :, 1:H + 1, 1:W + 1] = x
    xpad = xpad.astype(ml_dtypes.bfloat16)
    in_maps = []
    for c in range(n_cores):
        n, s = c // 2, c % 2
        r0 = s * HL if exchange else 0
        xc = np.ascontiguousarray(xpad[:, n, :, r0:r0 + HL + 2, :])
        if exchange:
            hm = np.array([[float(s == 1), float(s == 0)]], np.float32)
        else:
            hm = np.zeros((1, 2), np.float32)
        hm = np.broadcast_to(hm, (128, 2)).copy()
        in_maps.append({"x": xc, "w": w, "gb": gb, "ident": ident, "hm": hm})
    return in_maps


_last_results = None


def assemble(per_core_results, T, N, H, W, HL, exchange):
    """Gather per-core "out" buffers into the full hs [T, N, 64, H, W]."""
    hs = np.empty((T, N, CIN, H, W), np.float32)
    for n in range(N):
        if exchange:
            for s in range(2):
                o = np.asarray(per_core_results[2 * n + s]["out"]).astype(np.float32)
                hs[:, n, :, s * HL:(s + 1) * HL, :] = o.reshape(T, CIN, HL, W)
        else:
            o = np.asarray(per_core_results[2 * n]["out"]).astype(np.float32)
            hs[:, n] = o.reshape(T, CIN, H, W)
    return hs


def kernel(x, Wx, bx, gamma, beta, Wh, exchange=True):
    """Full-input entry point: returns hs [T, N, 64, H, W] float32."""
    global _last_results
    T, N, _, H, W = np.asarray(x).shape
    n_cores = 2 * N
    HL = H // 2 if exchange else H
    nc = build(T=T, HL=HL, W=W, n_cores=n_cores, exchange=exchange)
    in_maps = make_in_maps(x, Wx, gamma, beta, Wh, HL, exchange, n_cores)
    import time as _time
    _t0 = _time.monotonic()
    res = run_bass_kernel_spmd(nc, in_maps, list(range(n_cores)))
    globals()["_last_spmd_s"] = _time.monotonic() - _t0
    _last_results = res
    return assemble(res.results, T, N, H, W, HL, exchange)

